# revision 11
# baseline (speedup 1.0000x reference)
"""Trainium2 Bass kernel for the diffusion-sampler importance-weight problem.

v2: all matmuls fp8e4 DoubleRow (0.5 cyc/row), elementwise split across
ACT/DVE/Pool, eps pre-scaled by s_t and fp8-quantized on host (4x less DMA).

Math (per batch element, per z-dim), derived from the reference:
  z_0 = sigma0 * eps0
  per step t (beta_f = beta[t], beta_b = roll(beta,1)[t]):
    hid   = relu(W1z @ z8 + c1 + te_t)          c1 = ctx @ W1[Z:] + b1 (host)
    u'    = W2' @ hid8                          W2' = W2 * dt
    z'    = a_t * z + (u' + eps't)              eps't = fp8(s_t * eps_t), host
    d_t   = k_t (z - c_t z')^2                  (device, exact fp32)
    e_t   = 0.5 (eps't/s_t)^2                   (host, from the same fp8 vals)
  terminal: d_T += 0.5 (z_T - mu)^2   (device); e += 0.5 eps0^2 (host)
  output = sum_z mean_b (e - d) + Z * const

Device layout: feature-major [Z=128 partitions, batch free]. fp8 enters only
matmul operands; the z trajectory, v, and all accumulated sums stay fp32.
Layer-1 DoubleRow packs K=256 as (z8, c1[h]) via a stepped slice of one
[128, 8, BS] fp8 tile; layer-2 DoubleRow packs hid-chunk pairs; eps enters
PSUM via lhsT=[0|I] DoubleRow pairing (z-slot, eps-slot).
"""
import math
import numpy as np

B, Z, H, HID, T = 16384, 128, 512, 512, 32
NCORES = 8
BS = B // NCORES          # 2048 batch rows per core
NT = BS // 512            # 4 n-tiles of 512
SQ5 = float(np.sqrt(0.5))

_cache: dict = {}
# relu engine per (h, n): a=ACT, d=DVE, p=Pool
RELU_SPLIT = {
    (0, 0): "a", (0, 1): "a", (0, 2): "a", (0, 3): "a",
    (1, 0): "a", (1, 1): "a", (1, 2): "a", (1, 3): "a",
    (2, 0): "d", (2, 1): "d", (2, 2): "d", (2, 3): "d",
    (3, 0): "d", (3, 1): "d", (3, 2): "d", (3, 3): "d",
}


def _build_module(nop=False):
    import concourse.tile as tile
    from concourse import bacc, mybir

    f32 = mybir.dt.float32
    f8 = mybir.dt.float8e4
    AF = mybir.ActivationFunctionType
    ALU = mybir.AluOpType
    DR = mybir.MatmulPerfMode.DoubleRow

    nc = bacc.Bacc("TRN2", target_bir_lowering=False, debug=False,
                   num_devices=NCORES)

    eps8d = nc.dram_tensor("eps8d", [T, 128, BS], f8, kind="ExternalInput").ap()
    zc8d = nc.dram_tensor("zc8d", [128, 8, BS], f8, kind="ExternalInput").ap()
    z0d = nc.dram_tensor("z0d", [128, BS], f32, kind="ExternalInput").ap()
    muT = nc.dram_tensor("muT", [128, BS], f32, kind="ExternalInput").ap()
    w18d = nc.dram_tensor("w18d", [128, 4, 2, 128], f8, kind="ExternalInput").ap()
    w28d = nc.dram_tensor("w28d", [128, 2, 2, 128], f8, kind="ExternalInput").ap()
    ezI8d = nc.dram_tensor("ezI8d", [128, 2, 128], f8, kind="ExternalInput").ap()
    tbld = nc.dram_tensor("tbld", [128, 160], f32, kind="ExternalInput").ap()
    tetd = nc.dram_tensor("tetd", [128, 128], f32, kind="ExternalInput").ap()
    outd = nc.dram_tensor("outd", [128, 4], f32, kind="ExternalOutput").ap()

    with tile.TileContext(nc) as tc:
        with (
            tc.tile_pool(name="const", bufs=1) as cpool,
            tc.tile_pool(name="state", bufs=1) as spool,
            tc.tile_pool(name="hid", bufs=3) as hpool,
            tc.tile_pool(name="scr", bufs=2) as scrp,
            tc.tile_pool(name="psH", bufs=5, space="PSUM") as psH,
            tc.tile_pool(name="psZ", bufs=3, space="PSUM") as psZ,
        ):
            if nop:
                out2 = spool.tile([128, 2], f32, tag="out2")
                nc.gpsimd.memset(out2[:], 0.0)
                nc.sync.dma_start(outd, out2[:])
            else:
                _emit(nc, tc, cpool, spool, hpool, scrp, psH, psZ,
                      f32, f8, AF, ALU, DR,
                      eps8d, zc8d, z0d, muT, w18d, w28d, ezI8d, tbld, tetd,
                      outd)

    nc.compile()
    return nc


def _emit(nc, tc, cpool, spool, hpool, scrp, psH, psZ,
          f32, f8, AF, ALU, DR,
          eps8d, zc8d, z0d, muT, w18d, w28d, ezI8d, tbld, tetd, outd):
    import concourse.mybir as mybir

    # ---- resident constants ----
    w18 = cpool.tile([128, 4, 2, 128], f8, tag="w18")
    for h in range(4):
        nc.sync.dma_start(w18[:, h, :, :], w18d[:, h, :, :])
    w28 = cpool.tile([128, 2, 2, 128], f8, tag="w28")
    for p in range(2):
        nc.sync.dma_start(w28[:, p, :, :], w28d[:, p, :, :])
    ezI8 = cpool.tile([128, 2, 128], f8, tag="ezI8")
    nc.sync.dma_start(ezI8[:], ezI8d)
    tbl = cpool.tile([128, 160], f32, tag="tbl")
    nc.sync.dma_start(tbl[:], tbld)
    tet = cpool.tile([128, 128], f32, tag="tet")
    nc.sync.dma_start(tet[:], tetd)

    # zc8 slots: 0,1 = z8 ping-pong; 2..5 = c1[h]; 6,7 = eps8 ping-pong
    zc8 = spool.tile([128, 8, BS], f8, tag="zc8")
    for s in range(8):
        nc.sync.dma_start(zc8[:, s, :], zc8d[:, s, :])
    zA = spool.tile([128, BS], f32, tag="zA")
    nc.sync.dma_start(zA[:], z0d)
    zB = spool.tile([128, BS], f32, tag="zB")
    slots_r0 = spool.tile([128, 17], f32, tag="slr0")
    slots_r1 = spool.tile([128, 17], f32, tag="slr1")
    slots_q0 = spool.tile([128, 17], f32, tag="slq0")
    slots_q1 = spool.tile([128, 17], f32, tag="slq1")
    nc.gpsimd.memset(slots_r1[:, 16:17], 0.0)
    nc.gpsimd.memset(slots_q1[:, 16:17], 0.0)
    zbuf = [zA, zB]

    # ---- main loop (fully unrolled) ----
    nc.sync.dma_start(zc8[:, 6, :], eps8d[0])
    for t in range(T):
        p = t % 2
        zin = zbuf[p]
        zout = zbuf[1 - p]
        ep_slot = 6 + p
        if t + 1 < T:  # prefetch next step's eps into the other slot
            nc.sync.dma_start(zc8[:, 7 - p, :], eps8d[t + 1])

        for n in range(NT):
            nsl = slice(n * 512, (n + 1) * 512)
            zps = psZ.tile([128, 512], f32, tag="zps")
            # eps' into PSUM: lhsT = [0 | I], rhs pairs (z-slot, eps-slot)
            step_e = ep_slot - p
            rhs_e = zc8[:, p:ep_slot + 1:step_e, nsl]
            nc.tensor.matmul(zps[:], lhsT=ezI8[:], rhs=rhs_e,
                             start=True, stop=False, perf_mode=DR)
            hs8 = hpool.tile([128, 4, 512], f8, tag="hs8")
            for h in range(4):
                hp = psH.tile([128, 512], f32, tag="hp")
                step_h = 2 + h - p
                rhs_h = zc8[:, p:2 + h + 1:step_h, nsl]
                nc.tensor.matmul(hp[:], lhsT=w18[:, h, :, :], rhs=rhs_h,
                                 start=True, stop=True, perf_mode=DR)
                tecol = tet[:, h * 32 + t: h * 32 + t + 1]
                eng = RELU_SPLIT[(h, n)]
                if eng == "a":
                    nc.scalar.activation(hs8[:, h, :], hp[:], AF.Relu,
                                         bias=tecol, scale=1.0)
                elif eng == "d":
                    nc.vector.tensor_scalar(hs8[:, h, :], hp[:], scalar1=tecol,
                                            scalar2=0.0, op0=ALU.add,
                                            op1=ALU.max)
                else:
                    nc.gpsimd.tensor_scalar(hs8[:, h, :], hp[:], scalar1=tecol,
                                            scalar2=0.0, op0=ALU.add,
                                            op1=ALU.max)
            nc.tensor.matmul(zps[:], lhsT=w28[:, 0, :, :], rhs=hs8[:, 0:2, :],
                             start=False, stop=False, perf_mode=DR)
            nc.tensor.matmul(zps[:], lhsT=w28[:, 1, :, :], rhs=hs8[:, 2:4, :],
                             start=False, stop=True, perf_mode=DR)
            # z' = a_t * z + (u' + eps')   [exact fp32 on DVE]
            nc.vector.scalar_tensor_tensor(
                zout[:, nsl], in0=zin[:, nsl], scalar=tbl[:, t:t + 1],
                in1=zps[:], op0=ALU.mult, op1=ALU.add)
            # z8 for next step (slot 1-p), per n-tile for pipelining
            nc.gpsimd.tensor_copy(zc8[:, 1 - p, nsl], zout[:, nsl])

        # d-term decomposition (no v tensor):
        #   r_t = sum w_t z_t^2      [ACT Square accum, scale=sqrt(w_t)]
        #   q_t = sum (-2 k_t c_t) z_t z_{t+1}   [DVE STT accum, mult-mult]
        sr = scrp.tile([128, 1], f32, tag="scrA")
        rslot = (slots_r0 if p == 0 else slots_r1)[:, t // 2:t // 2 + 1]
        nc.scalar.activation(
            sr[:].broadcast_to((128, BS)), zin[:], AF.Square, bias=0.0,
            scale=tbl[:, 97 + t:98 + t], accum_out=rslot)
        sq = scrp.tile([128, 1], f32, tag="scrB")
        qslot = (slots_q0 if p == 0 else slots_q1)[:, t // 2:t // 2 + 1]
        nc.vector.scalar_tensor_tensor(
            sq[:].broadcast_to((128, BS)), in0=zin[:],
            scalar=tbl[:, 64 + t:65 + t], in1=zout[:],
            op0=ALU.mult, op1=ALU.mult, accum_out=qslot)

    # ---- terminal: w_32 z_T^2 and -z_T.mu  (0.5 mu^2 handled on host) ----
    zfin = zbuf[T % 2]
    mu = scrp.tile([128, BS], f32, tag="scrV")
    nc.sync.dma_start(mu[:], muT)
    sr = scrp.tile([128, 1], f32, tag="scrA")
    nc.scalar.activation(
        sr[:].broadcast_to((128, BS)), zfin[:], AF.Square, bias=0.0,
        scale=tbl[:, 97 + T:98 + T], accum_out=slots_r0[:, 16:17])
    sq = scrp.tile([128, 1], f32, tag="scrB")
    nc.vector.scalar_tensor_tensor(
        sq[:].broadcast_to((128, BS)), in0=zfin[:], scalar=-1.0, in1=mu[:],
        op0=ALU.mult, op1=ALU.mult, accum_out=slots_q0[:, 16:17])

    out2 = spool.tile([128, 4], f32, tag="out2")
    nc.vector.tensor_reduce(out2[:, 0:1], slots_r0[:],
                            axis=mybir.AxisListType.X, op=ALU.add)
    nc.vector.tensor_reduce(out2[:, 1:2], slots_r1[:],
                            axis=mybir.AxisListType.X, op=ALU.add)
    nc.vector.tensor_reduce(out2[:, 2:3], slots_q0[:],
                            axis=mybir.AxisListType.X, op=ALU.add)
    nc.vector.tensor_reduce(out2[:, 3:4], slots_q1[:],
                            axis=mybir.AxisListType.X, op=ALU.add)
    nc.sync.dma_start(outd, out2[:])


def _host_prep(inputs):
    """Numpy-only preprocessing: transposes, fp8 quantization, scalar tables,
    host-side e-term sums (from the same fp8 values the device consumes)."""
    import ml_dtypes
    F8 = ml_dtypes.float8_e4m3

    ctx = np.asarray(inputs["context_embedding"], np.float32)
    eps0 = np.asarray(inputs["eps0"], np.float32)
    eps = np.asarray(inputs["eps"], np.float32)
    beta = np.asarray(inputs["beta_schedule"], np.float64)
    sig0 = float(np.asarray(inputs["sigma0"], np.float32)[0])
    W1 = np.asarray(inputs["W1"], np.float32)
    b1 = np.asarray(inputs["b1"], np.float32)
    W2 = np.asarray(inputs["W2"], np.float32)
    b2 = np.asarray(inputs["b2"], np.float32)
    te = np.asarray(inputs["t_emb"], np.float32)
    mu = np.asarray(inputs["target_mu"], np.float32)

    dt = 1.0 / T
    bb = np.roll(beta, 1)
    a_t = (1.0 + beta * dt).astype(np.float32)
    c_t = (1.0 - bb * dt).astype(np.float32)
    s_t = (np.sqrt(2.0 * beta * dt) * sig0).astype(np.float32)
    sb_t = (np.sqrt(2.0 * bb * dt) * sig0).astype(np.float32)
    k_t = (0.5 / (sb_t.astype(np.float64) ** 2)).astype(np.float32)

    if np.any(b2):
        raise NotImplementedError("nonzero b2 not supported by this kernel")

    c1 = (ctx @ W1[Z:] + b1).astype(np.float32)           # [B, HID]
    c1_T = np.ascontiguousarray(c1.T).astype(F8)          # [HID, B] fp8
    eps_T = eps.transpose(0, 2, 1)                        # [T, Z, B]
    eps_s = (eps_T * s_t[:, None, None]).astype(F8)       # fp8(s_t * eps)
    z0 = (sig0 * eps0.T).astype(np.float32)               # [Z, B]
    mu_T = np.ascontiguousarray(mu.T)                     # [Z, B]

    # host e-term: 0.5 * (deq(eps8)/s_t)^2 summed over z and t, per batch col
    deq = eps_s.astype(np.float32) / s_t[:, None, None]
    e_sum = float(0.5 * np.sum(deq.astype(np.float64) ** 2))
    e_sum += float(0.5 * np.sum(eps0.astype(np.float64) ** 2))
    e_sum -= float(0.5 * np.sum(mu.astype(np.float64) ** 2))

    ident = np.eye(128, dtype=np.float32)
    w18 = np.zeros((128, 4, 2, 128), np.float32)
    for h in range(4):
        w18[:, h, 0, :] = W1[:Z, h * 128:(h + 1) * 128]
        w18[:, h, 1, :] = ident
    w18 = w18.astype(F8)
    w2dt = (W2 * np.float32(dt)).astype(np.float32)       # [HID, Z]
    w28 = np.zeros((128, 2, 2, 128), np.float32)
    for p in range(2):
        for s in range(2):
            w28[:, p, s, :] = w2dt[(2 * p + s) * 128:(2 * p + s + 1) * 128, :]
    w28 = w28.astype(F8)
    ezI8 = np.zeros((128, 2, 128), np.float32)
    ezI8[:, 1, :] = ident
    ezI8 = ezI8.astype(F8)

    # d-term decomposition coefficients:
    #   sum_t k_t (z_t - c_t z_{t+1})^2 + 0.5 (z_T - mu)^2
    # = sum_{t=0..32} w_t z_t^2 + sum_t (-2 k_t c_t) z_t z_{t+1} - z_T.mu + 0.5 mu^2
    k64 = k_t.astype(np.float64)
    c64 = c_t.astype(np.float64)
    w = np.zeros(T + 1)
    w[0:T] += k64
    w[1:T + 1] += k64 * c64 * c64
    w[T] += 0.5
    qc = (-2.0 * k64 * c64).astype(np.float32)
    tbl = np.zeros((128, 160), np.float32)
    tbl[:, 0:T] = a_t[None, :]
    tbl[:, 64:64 + T] = qc[None, :]
    tbl[:, 97:97 + T + 1] = np.sqrt(w).astype(np.float32)[None, :]

    tet = np.zeros((128, 128), np.float32)
    for h in range(4):
        tet[:, h * 32:(h + 1) * 32] = te[:, h * 128:(h + 1) * 128].T

    const = float(np.sum(np.log(s_t.astype(np.float64))
                         - np.log(sb_t.astype(np.float64))) + math.log(sig0))

    zc8 = np.zeros((128, 8, B), np.float32)
    zc8[:, 0, :] = z0
    zc8[:, 2:6, :] = c1_T.astype(np.float32).reshape(4, 128, B).transpose(1, 0, 2)
    zc8 = zc8.astype(F8)

    in_maps = []
    for c in range(NCORES):
        bs = slice(c * BS, (c + 1) * BS)
        in_maps.append({
            "eps8d": np.ascontiguousarray(eps_s[:, :, bs]),
            "zc8d": np.ascontiguousarray(zc8[:, :, bs]),
            "z0d": np.ascontiguousarray(z0[:, bs]),
            "muT": np.ascontiguousarray(mu_T[:, bs]),
            "w18d": w18,
            "w28d": w28,
            "ezI8d": ezI8,
            "tbld": tbl,
            "tetd": tet,
        })
    return in_maps, const, e_sum


def _install_neff_cache():
    """Cache walrus NEFF output by BIR hash."""
    import hashlib
    import os
    import shutil

    from concourse import bass2jax

    if getattr(bass2jax, "_ant_neff_cache_installed", False):
        return
    orig = bass2jax.compile_bir_kernel
    cache_dir = os.environ.get("BASS_NEFF_CACHE", "/tmp/neff_cache")

    def cached(bir_json, tmpdir, neff_name="file.neff"):
        os.makedirs(cache_dir, exist_ok=True)
        key = hashlib.sha256(bir_json if isinstance(bir_json, bytes)
                             else bir_json.encode()).hexdigest()[:24]
        hit = os.path.join(cache_dir, f"{key}.neff")
        dst = os.path.join(tmpdir, neff_name)
        if os.path.exists(hit):
            shutil.copy(hit, dst)
            return dst
        out = orig(bir_json, tmpdir, neff_name)
        shutil.copy(out, hit)
        return out

    bass2jax.compile_bir_kernel = cached
    bass2jax._ant_neff_cache_installed = True


def kernel(**inputs) -> np.ndarray:
    from concourse import bass_utils

    _install_neff_cache()
    if "nc" not in _cache:
        _cache["nc"] = _build_module()
    nc = _cache["nc"]

    in_maps, const, e_sum = _host_prep(inputs)
    res = bass_utils.run_bass_kernel_spmd(nc, in_maps, core_ids=list(range(NCORES)))
    _cache["last_res"] = res
    d_sum = 0.0
    for c in range(NCORES):
        o = res.results[c]["outd"].astype(np.float64)
        d_sum += float(np.sum(o))
    total = (e_sum - d_sum) / B + Z * const
    return np.float32(total)
